# revision 1
# baseline (speedup 1.0000x reference)
"""Trainium2 Bass kernel for the MHA+LayerNorm block (B=4,S=2048,D=768,H=12,E=64).

Sharding: 8 cores = 4 batches x 2 query-halves. Each core computes 1024 query
rows of one batch against the full 2048-key sequence. Zero collectives.

All cores run ONE identical NEFF. Per-core input rows are permuted on the host
so that the core's own query half is always rows [0:1024) of `x` (attention is
a sum over t, invariant to key/value permutation as long as the mask rows are
permuted identically).
"""

import numpy as np
import ml_dtypes

from contextlib import ExitStack

import concourse.bass as bass
import concourse.tile as tile
from concourse import bacc, mybir
from concourse import bass_utils

B, S, D = 4, 2048, 768
H, E = 12, 64
HE = H * E          # 768
SQ = 1024           # query rows per core
N_CORES = 8
SCALE = 1.0 / float(np.sqrt(S))
LN_EPS = 1e-5

F32 = mybir.dt.float32
F32R = mybir.dt.float32r
BF16 = mybir.dt.bfloat16

NKT = D // 128      # 6 contraction tiles over d
NKB = HE // 128     # 6 head-pair blocks
NTT = S // 128      # 16 key tiles
NSB = SQ // 128     # 8 query blocks
VW = H * (E + 1)    # 780: per-head 64 V columns + 1 ones column

LAST_EXEC_NS = None
_NC_CACHE = {}


def _bcast_ap(ap, parts):
    return bass.AP(tensor=ap.tensor, offset=ap.offset, ap=[[0, parts], list(ap.ap[-1])])


def _build_nc(trivial_ln=True):
    nc = bacc.Bacc(None, target_bir_lowering=False)

    x_d = nc.dram_tensor("x", [D, S], BF16, kind="ExternalInput")  # pre-transposed on host
    multT_d = nc.dram_tensor("multT", [S, SQ], BF16, kind="ExternalInput")
    wq_d = nc.dram_tensor("wq", [D, HE], BF16, kind="ExternalInput")
    wk_d = nc.dram_tensor("wk", [D, HE], BF16, kind="ExternalInput")
    wv_d = nc.dram_tensor("wv", [D, VW], BF16, kind="ExternalInput")
    bq_d = nc.dram_tensor("bq", [128, NKB], F32, kind="ExternalInput")
    bk_d = nc.dram_tensor("bk", [128, NKB], F32, kind="ExternalInput")
    bv_d = nc.dram_tensor("bv", [1, VW], BF16, kind="ExternalInput")
    wo_d = nc.dram_tensor("wo", [HE, D], BF16, kind="ExternalInput")
    bo_d = nc.dram_tensor("bo", [1, D], F32, kind="ExternalInput")
    gamma_d = nc.dram_tensor("gamma", [1, D], F32, kind="ExternalInput")
    beta_d = nc.dram_tensor("beta", [1, D], F32, kind="ExternalInput")
    out_d = nc.dram_tensor("out", [SQ, D], F32, kind="ExternalOutput")

    Exp = mybir.ActivationFunctionType.Exp
    Sqrt = mybir.ActivationFunctionType.Sqrt
    Ident = mybir.ActivationFunctionType.Identity

    with tile.TileContext(nc) as tc, ExitStack() as ctx:
        persist = ctx.enter_context(tc.tile_pool(name="persist", bufs=1))
        qt = [persist.tile([128, SQ], BF16, name=f"qt{i}", tag=f"qt{i}") for i in range(NKB)]
        kt = [persist.tile([128, S], BF16, name=f"kt{i}", tag=f"kt{i}") for i in range(NKB)]
        vaug = [persist.tile([128, VW], BF16, name=f"va{i}", tag=f"va{i}") for i in range(NTT)]
        ctxh = [persist.tile([128, SQ], BF16, name=f"cx{i}", tag=f"cx{i}") for i in range(NKB)]
        multT = [persist.tile([128, SQ], BF16, name=f"mT{i}", tag=f"mT{i}") for i in range(NTT)]
        wo_sb = [persist.tile([128, D], BF16, name=f"wo{i}", tag=f"wo{i}") for i in range(NKB)]
        xt = [persist.tile([128, S], BF16, name=f"xt{i}", tag=f"xt{i}") for i in range(NKT)]
        bq_sb = persist.tile([128, NKB], F32, name="bq_sb", tag="bq_sb")
        bk_sb = persist.tile([128, NKB], F32, name="bk_sb", tag="bk_sb")
        # DMA issue order = consumption order: x (V matmuls, immediately),
        # biases (first QK evac ~30us in), mask tiles (attention loop),
        # wo last (phase 3 only)
        for i in range(NKT):
            nc.sync.dma_start(out=xt[i], in_=x_d[i * 128:(i + 1) * 128, :])
        nc.sync.dma_start(out=bq_sb, in_=bq_d[:, :])
        nc.sync.dma_start(out=bk_sb, in_=bk_d[:, :])

        wsp = ctx.enter_context(tc.tile_pool(name="ws", bufs=24))

        def load_w(kb2):
            tiles = []
            for w_d in (wq_d, wk_d):
                for i in range(NKT):
                    w = wsp.tile([128, 128], BF16, name="w", tag="ws")
                    nc.sync.dma_start(
                        out=w, in_=w_d[i * 128:(i + 1) * 128, kb2 * 128:(kb2 + 1) * 128])
                    tiles.append(w)
            return tiles

        # ---------------- Phase 1a: V (natural, with per-head ones column)
        with tc.tile_pool(name="p1", bufs=1) as p1, \
             tc.tile_pool(name="vps", bufs=2, space="PSUM") as vp:
            wv_sb = [p1.tile([128, VW], BF16, name=f"wv{i}", tag=f"wv{i}") for i in range(NKT)]
            bv_bc = p1.tile([128, VW], BF16, name="bv_bc", tag="bv_bc")
            nc.sync.dma_start(out=bv_bc, in_=_bcast_ap(bv_d[:, :], 128))
            for i in range(NKT):
                nc.sync.dma_start(out=wv_sb[i], in_=wv_d[i * 128:(i + 1) * 128, :])
            # later-phase loads issued in consumption order so they never
            # delay the V-phase weights: qk weights for block 0, mask tiles,
            # then wo (phase 3 only)
            wt0 = load_w(0)
            for t in range(NTT):
                nc.sync.dma_start(out=multT[t], in_=multT_d[t * 128:(t + 1) * 128, :])
            for i in range(NKB):
                nc.sync.dma_start(out=wo_sb[i], in_=wo_d[i * 128:(i + 1) * 128, :])
            for t in range(NTT):
                psv = vp.tile([128, VW], F32, name="psv", tag="psv")
                for i in range(NKT):
                    st, sp = (i == 0), (i == NKT - 1)
                    lhsT = xt[i][:, t * 128:(t + 1) * 128]
                    nc.tensor.matmul(psv[:, 0:512], lhsT, wv_sb[i][:, 0:512],
                                     start=st, stop=sp)
                    nc.tensor.matmul(psv[:, 512:VW], lhsT, wv_sb[i][:, 512:VW],
                                     start=st, stop=sp)
                nc.vector.tensor_add(vaug[t], psv, bv_bc)

        # ---------------- Main loop: QK projection (kb+1) interleaved with
        # attention (kb). PSUM: qk chunks 2x1 + scores 2x2 + ctx 1x2 = 8 banks.
        with tc.tile_pool(name="attnp", bufs=4) as attnp, \
             tc.tile_pool(name="rp", bufs=2) as rp, \
             tc.tile_pool(name="cxp", bufs=2) as cxp, \
             tc.tile_pool(name="qkp", bufs=2, space="PSUM") as qkp, \
             tc.tile_pool(name="sps", bufs=2, space="PSUM") as sps, \
             tc.tile_pool(name="cps", bufs=1, space="PSUM") as cps, \
             tc.tile_pool(name="drp", bufs=4, space="DRAM") as drp:

            def emit_qk_chunk(kb2, wt, c):
                # c 0-1: Q chunks (SQ = 2x512); c 2-5: K chunks (S = 4x512)
                if c < 2:
                    dst, bias, off, ws = qt[kb2], bq_sb, c * 512, wt[0:NKT]
                else:
                    dst, bias, off, ws = kt[kb2], bk_sb, (c - 2) * 512, wt[NKT:2 * NKT]
                pq = qkp.tile([128, 512], F32, name="pq", tag="qk")
                for i in range(NKT):
                    nc.tensor.matmul(pq, ws[i], xt[i][:, off:off + 512],
                                     start=(i == 0), stop=(i == NKT - 1))
                nc.vector.tensor_scalar_add(dst[:, off:off + 512], pq,
                                            bias[:, kb2:kb2 + 1])

            for c in range(6):
                emit_qk_chunk(0, wt0, c)

            for kb in range(NKB):
                wt_next = load_w(kb + 1) if kb < NKB - 1 else None
                for half in range(2):
                    h = 2 * kb + half
                    p0 = 64 * half
                    cpsum = cps.tile([128, SQ], F32, name="ctx", tag="ctx")
                    attns = []

                    def emit_ctx(tt):
                        st, sp = (tt == 0), (tt == NTT - 1)
                        for chs in range(0, SQ, 512):
                            nc.tensor.matmul(cpsum[0:65, chs:chs + 512],
                                             vaug[tt][:, h * 65:(h + 1) * 65],
                                             attns[tt][:, chs:chs + 512],
                                             start=st, stop=sp)

                    for t in range(NTT):
                        ps = sps.tile([128, SQ], F32, name="ps", tag="ps")
                        kl = kt[kb][p0:p0 + 64, t * 128:(t + 1) * 128]
                        for chs in range(0, SQ, 512):
                            nc.tensor.matmul(ps[:, chs:chs + 512], kl,
                                             qt[kb][p0:p0 + 64, chs:chs + 512],
                                             start=True, stop=True)
                        attn = attnp.tile([128, SQ], BF16, name="attn", tag="attn")
                        nc.scalar.activation(attn, ps, Exp, scale=SCALE)
                        nc.vector.tensor_mul(attn, attn, multT[t])
                        attns.append(attn)
                        if t > 0:
                            emit_ctx(t - 1)
                        if kb < NKB - 1 and t in (5, 10, 15):
                            emit_qk_chunk(kb + 1, wt_next, 3 * half + (5, 10, 15).index(t))
                    emit_ctx(NTT - 1)

                    # evacuate ctx+denominator fast to free the PSUM bank,
                    # then normalize off the critical path
                    recip = rp.tile([1, SQ], F32, name="recip", tag="recip")
                    nc.vector.reciprocal(recip, cpsum[64:65, :])
                    rb_d = drp.tile([1, SQ], F32, name="rb_d", tag="rb")
                    nc.sync.dma_start(out=rb_d, in_=recip)
                    cxu = cxp.tile([64, SQ], F32, name="cxu", tag="cxu")
                    nc.vector.tensor_scalar_add(cxu, cpsum[0:64, :], 0.0)
                    rbc = rp.tile([64, SQ], F32, name="rbc", tag="rbc")
                    nc.sync.dma_start(out=rbc, in_=_bcast_ap(rb_d, 64))
                    nc.vector.tensor_mul(ctxh[kb][p0:p0 + 64, :], cxu, rbc)

        # ---------------- Phase 3: output projection + LayerNorm
        with tc.tile_pool(name="p3", bufs=1) as p3, \
             tc.tile_pool(name="op", bufs=6) as op, \
             tc.tile_pool(name="lnp", bufs=8) as lnp, \
             tc.tile_pool(name="ops", bufs=4, space="PSUM") as ops:
            bo_bc = p3.tile([128, D], F32, name="bo_bc", tag="bo_bc")
            eps_sb = p3.tile([128, 1], F32, name="eps_sb", tag="eps_sb")
            nc.vector.memset(eps_sb, LN_EPS)
            nc.sync.dma_start(out=bo_bc, in_=_bcast_ap(bo_d[:, :], 128))
            if not trivial_ln:
                gamma_bc = p3.tile([128, D], F32, name="gamma_bc", tag="gamma_bc")
                beta_bc = p3.tile([128, D], F32, name="beta_bc", tag="beta_bc")
                nc.sync.dma_start(out=gamma_bc, in_=_bcast_ap(gamma_d[:, :], 128))
                nc.sync.dma_start(out=beta_bc, in_=_bcast_ap(beta_d[:, :], 128))

            for sb in range(NSB):
                pso = ops.tile([128, D], F32, name="pso", tag="pso")
                for i in range(NKB):
                    lhsT = ctxh[i][:, sb * 128:(sb + 1) * 128]
                    nc.tensor.matmul(pso[:, 0:512], lhsT, wo_sb[i][:, 0:512],
                                     start=(i == 0), stop=(i == NKB - 1))
                    nc.tensor.matmul(pso[:, 512:D], lhsT, wo_sb[i][:, 512:D],
                                     start=(i == 0), stop=(i == NKB - 1))

                o_sb = op.tile([128, D], F32, name="o_sb", tag="o_sb")
                nc.vector.tensor_add(o_sb, pso, bo_bc)

                stats = lnp.tile([128, 3, 6], F32, name="stats", tag="stats")
                mv = lnp.tile([128, 2], F32, name="mv", tag="mv")
                o_rs = o_sb.rearrange("p (n f) -> p n f", f=256)
                for g in range(3):
                    nc.vector.bn_stats(out=stats[:, g, :], in_=o_rs[:, g, :])
                nc.vector.bn_aggr(out=mv, in_=stats)
                std = lnp.tile([128, 1], F32, name="std", tag="std")
                nc.scalar.activation(out=std, in_=mv[:, 1:2], func=Sqrt, bias=eps_sb)
                nc.vector.reciprocal(out=std, in_=std)
                nc.vector.tensor_scalar(out=o_sb, in0=o_sb, scalar1=mv[:, 0:1],
                                        scalar2=std, op0=mybir.AluOpType.subtract,
                                        op1=mybir.AluOpType.mult)
                if not trivial_ln:
                    nc.vector.tensor_mul(o_sb, o_sb, gamma_bc)
                    nc.vector.tensor_add(o_sb, o_sb, beta_bc)
                nc.sync.dma_start(out=out_d[sb * 128:(sb + 1) * 128, :], in_=o_sb)

    nc.finalize()
    return nc


def _get_nc(trivial_ln=True):
    if trivial_ln not in _NC_CACHE:
        _NC_CACHE[trivial_ln] = _build_nc(trivial_ln)
    return _NC_CACHE[trivial_ln]


def build_in_maps(inputs):
    x = np.asarray(inputs["input_tensor"], np.float32)       # [B,S,D]
    mask = np.asarray(inputs["attention_mask"])              # [B,S,S] bool
    Wq = np.asarray(inputs["Wq"], np.float32)                # [H,D,E]
    bq = np.asarray(inputs["bq"], np.float32)                # [H,E]
    Wk = np.asarray(inputs["Wk"], np.float32)
    bk = np.asarray(inputs["bk"], np.float32)
    Wv = np.asarray(inputs["Wv"], np.float32)
    bv = np.asarray(inputs["bv"], np.float32)
    Wo = np.asarray(inputs["Wo"], np.float32)                # [HE,D]
    bo = np.asarray(inputs["bo"], np.float32)                # [D]
    gamma = np.asarray(inputs["gamma"], np.float32)
    beta = np.asarray(inputs["beta"], np.float32)

    bf = ml_dtypes.bfloat16
    wq_mat = np.ascontiguousarray(Wq.transpose(1, 0, 2).reshape(D, HE)).astype(bf)
    wk_mat = np.ascontiguousarray(Wk.transpose(1, 0, 2).reshape(D, HE)).astype(bf)
    # V weights with a ones/bias augmentation column per head (col h*65+64)
    wv_mat = np.zeros((D, VW), np.float32)
    bv_row = np.zeros((1, VW), np.float32)
    for h in range(H):
        wv_mat[:, h * 65:h * 65 + 64] = Wv[h]
        bv_row[0, h * 65:h * 65 + 64] = bv[h]
        bv_row[0, h * 65 + 64] = 1.0
    wv_mat = wv_mat.astype(bf)
    bv_row = bv_row.astype(bf)
    bq_col = np.ascontiguousarray(bq.reshape(NKB, 128).T).astype(np.float32)
    bk_col = np.ascontiguousarray(bk.reshape(NKB, 128).T).astype(np.float32)
    wo_bf = np.ascontiguousarray(Wo).astype(ml_dtypes.bfloat16)
    bo_row = bo.reshape(1, D).astype(np.float32)
    gamma_row = np.ascontiguousarray(gamma.reshape(1, D))
    beta_row = np.ascontiguousarray(beta.reshape(1, D))

    in_maps = []
    for c in range(N_CORES):
        b, qh = c // 2, c % 2
        sq0 = qh * SQ
        perm = np.concatenate([np.arange(sq0, sq0 + SQ), np.arange(0, sq0),
                               np.arange(sq0 + SQ, S)]).astype(np.int64)
        x_in = np.ascontiguousarray(x[b][perm].T).astype(bf)   # [D, S]
        m = (~mask[b][sq0:sq0 + SQ, :]).astype(np.float32)   # [SQ, S]
        multT = np.ascontiguousarray(m[:, perm].T).astype(bf)
        in_maps.append({
            "x": x_in, "multT": multT,
            "wq": wq_mat, "wk": wk_mat, "wv": wv_mat,
            "bq": bq_col, "bk": bk_col, "bv": bv_row,
            "wo": wo_bf, "bo": bo_row,
            "gamma": gamma_row, "beta": beta_row,
        })
    return in_maps


def kernel(**inputs):
    global LAST_EXEC_NS
    import os

    in_maps = build_in_maps(inputs)
    trivial_ln = bool(np.all(np.asarray(inputs["gamma"]) == 1.0)
                      and np.all(np.asarray(inputs["beta"]) == 0.0))
    nc = _get_nc(trivial_ln)
    trace = os.environ.get("BASS_MHA_TRACE", "0") == "1"
    res = bass_utils.run_bass_kernel_spmd(nc, in_maps, core_ids=list(range(N_CORES)),
                                          trace=trace)
    LAST_EXEC_NS = res.exec_time_ns

    out = np.empty((B, S, D), np.float32)
    for c in range(N_CORES):
        b, qh = c // 2, c % 2
        out[b, qh * SQ:(qh + 1) * SQ] = np.asarray(res.results[c]["out"], np.float32)
    return out

